# revision 30
# baseline (speedup 1.0000x reference)
"""Trainium2 Bass kernel for nn_C_BatchNorm (complex batch-norm, training mode).

Problem: z [B=32, C=128, H=64, W=64, 2] fp32.  Per position n=(c,h,w):
  2x2 covariance over batch, closed-form inverse sqrt, whiten, gamma/beta.

Sharding: C split across 8 cores (16 channels each).  Per core the shard is
[32, 131072] fp32 (16 MiB in / 16 MiB out), processed as 32 tiles of
[128 partitions = 4 position-groups x 32 batch, 1024 = 512 positions x 2
interleaved components].

v2 design (bf16 resident, de-interleaved):
  Phase 1 (per tile): DMA f32 tile -> staging; ACT copy de-interleaves and
    casts to bf16 resident zb (z0|z1 halves); DVE computes zz=zb*zb and
    zx=zb0*zb1 in bf16 2x mode; 5 bf16 matmuls with per-tile [128,128]
    selectors accumulate S|Q|X moments for ALL tiles directly into PSUM
    rows 32*(t//8) + 4*(t%8) + j -- no staging, no repack.
  Phase 2 (once): closed-form 2x2 inverse-sqrt + gamma fold on de-interleaved
    [128,512] planes straight from PSUM -> bf16 coefficient planes
    Pap=(A00|A10), Qap=(A01|A11), Rap=(R0|R1) where out_i = Ai0 z0 + Ai1 z1
    + Ri.
  Phase 3 (per tile): 6 bf16 indicator matmuls broadcast the tile's 4
    coefficient rows to 128 partitions in PSUM; ACT casts P/Q broadcast to
    bf16 SBUF; DVE (2x mode) multiplies with rep-views of zb and adds;
    Pool adds the R broadcast (PSUM) and re-interleaves into the f32 output
    tile, which is DMA'd out.
"""

import numpy as np
import ml_dtypes

import concourse.bass as bass
import concourse.bacc as bacc
import concourse.tile as tile
from concourse import mybir
from concourse.bass_utils import run_bass_kernel_spmd

f32 = mybir.dt.float32
bf16 = mybir.dt.bfloat16
f32r = mybir.dt.float32r
AF = mybir.ActivationFunctionType
OP = mybir.AluOpType

# ---- problem geometry (hardcoded) ----
B, C, H, W = 32, 128, 64, 64
NCORES = 8
C_PER = C // NCORES                  # 16 channels per core
NPOS = C_PER * H * W                 # 65536 positions per core
M = NPOS * 2                         # 131072 fp32 per batch row per core
NT = 32                              # tiles per core
FP = 512                             # positions per group per tile
COLS = 2 * FP                        # 1024 fp32 per partition per tile
J = 4                                # position groups per tile (32 batch each)
NB = 512                             # matmul free-dim chunk (one PSUM bank)


def _host_constants():
    # sel32[t]: [128, 128] with sel[p, w] = 1 iff w == 32*(t//8)+4*(t%8)+p//32
    sel = np.zeros((128, NT, 128), dtype=np.float32)
    for t in range(NT):
        g, i = divmod(t, 8)
        for p in range(128):
            sel[p, t, 32 * g + 4 * i + p // 32] = 1.0
    sel = sel.reshape(128, NT * 128).astype(ml_dtypes.bfloat16)
    # ind for phase-3 broadcast: 8 variants [128, 128] (row pattern mod 32):
    # ind[p, 128*i + q] = 1 iff (p % 32) == 4*i + q//32
    ind = np.zeros((128, 8, 128), dtype=np.float32)
    for i in range(8):
        for p in range(128):
            for jj in range(4):
                if p % 32 == 4 * i + jj:
                    ind[p, i, 32 * jj:32 * (jj + 1)] = 1.0
    ind = ind.reshape(128, 8 * 128).astype(ml_dtypes.bfloat16)
    return sel, ind


def build_module(reps=1):
    nc = bacc.Bacc("TRN2", target_bir_lowering=False, debug=False,
                   detect_race_conditions=False)
    z_d = nc.dram_tensor("z", [B, M], f32, kind="ExternalInput").ap()
    gamma_d = nc.dram_tensor("gamma", [2, 2], f32, kind="ExternalInput").ap()
    beta_d = nc.dram_tensor("beta", [2], f32, kind="ExternalInput").ap()
    sel_d = nc.dram_tensor("sel32", [128, NT * 128], bf16,
                           kind="ExternalInput").ap()
    ind_d = nc.dram_tensor("ind", [128, 8 * 128], bf16,
                           kind="ExternalInput").ap()
    ident_d = nc.dram_tensor("ident", [128, 128], bf16,
                             kind="ExternalInput").ap()
    out_d = nc.dram_tensor("out", [B, M], f32, kind="ExternalOutput").ap()

    # DRAM views ordered [tile, group, batch, col] (partition p = 32*j + b)
    z_r = z_d.rearrange("b (t j f) -> t j b f", t=NT, j=J, f=COLS)
    out_r = out_d.rearrange("b (t j f) -> t j b f", t=NT, j=J, f=COLS)

    irB = 1.0 / np.sqrt(np.float32(B))      # 1/sqrt(B)

    with tile.TileContext(nc) as tc:
        with (
            tc.tile_pool(name="consts", bufs=1) as consts,
            tc.tile_pool(name="zres", bufs=1) as zres,
            tc.tile_pool(name="ph2", bufs=1) as ph2,
            tc.tile_pool(name="work", bufs=3) as work,
            tc.tile_pool(name="work2", bufs=3) as work2,
            tc.tile_pool(name="obpool", bufs=3) as obpool,
        ):
            # ---------- constants ----------
            sel_sb = consts.tile([128, NT * 128], bf16)
            nc.sync.dma_start(out=sel_sb[:], in_=sel_d)
            ind_sb = consts.tile([128, 8 * 128], bf16)
            nc.sync.dma_start(out=ind_sb[:], in_=ind_d)
            ident_sb = consts.tile([128, 128], bf16)
            nc.sync.dma_start(out=ident_sb[:], in_=ident_d)

            gcols = consts.tile([128, 6], f32)   # g00 g01 g10 g11 b0 b1
            for k in range(4):
                nc.gpsimd.dma_start(
                    out=gcols[:, k:k + 1],
                    in_=bass.AP(tensor=gamma_d.tensor, offset=k,
                                ap=[[0, 128], [1, 1]]))
            for k in range(2):
                nc.gpsimd.dma_start(
                    out=gcols[:, 4 + k:5 + k],
                    in_=bass.AP(tensor=beta_d.tensor, offset=k,
                                ap=[[0, 128], [1, 1]]))
            # preload ACT spline tables (Square/Sqrt) during phase 0
            warm = consts.tile([128, 2], f32)
            nc.scalar.square(warm[:, 0:1], gcols[:, 0:1])
            nc.scalar.activation(warm[:, 1:2], warm[:, 0:1], AF.Sqrt)
            g00c, g01c = gcols[:, 0:1], gcols[:, 1:2]
            g10c, g11c = gcols[:, 2:3], gcols[:, 3:4]
            b0c, b1c = gcols[:, 4:5], gcols[:, 5:6]

            # resident z for the whole core, bf16, de-interleaved per tile:
            # tile t occupies cols [t*COLS, t*COLS+FP) = z0, [+FP, +COLS) = z1
            zb_all = zres.tile([128, NT * COLS], bf16)

            def _pipeline():
                tc.tile_update_base_wait()
                # ---------- phase 1: moments straight into PSUM ----------
                with tc.tile_pool(name="psum1", bufs=1, space="PSUM") as psum1:
                    ps_S = psum1.tile([128, COLS], f32)
                    ps_Q = psum1.tile([128, COLS], f32)
                    ps_X = psum1.tile([128, FP], f32)
                    for t in range(NT):
                        zf = work.tile([128, COLS], f32, tag="zf")
                        nc.sync.dma_start(out=zf[:], in_=z_r[t])
                        zb = zb_all[:, t * COLS:(t + 1) * COLS]
                        # de-interleave + cast: zb[c*FP + n] = zf[2n + c]
                        zb_v = bass.AP(tensor=zb_all.tensor,
                                       offset=t * COLS,
                                       ap=[list(zb_all.ap[0]), [FP, 2], [1, FP]])
                        zf_v = bass.AP(tensor=zf.tensor, offset=zf.offset,
                                       ap=[list(zf.ap[0]), [1, 2], [2, FP]])
                        nc.scalar.copy(zb_v, zf_v)
                        zb0 = zb_all[:, t * COLS:t * COLS + FP]
                        zb1 = zb_all[:, t * COLS + FP:(t + 1) * COLS]
                        zzb = work.tile([128, COLS], bf16, tag="zz")
                        nc.vector.tensor_tensor(zzb[:], zb, zb, OP.mult)
                        zxb = work.tile([128, FP], bf16, tag="zx")
                        nc.vector.tensor_tensor(zxb[:], zb0, zb1, OP.mult)

                        lhs = sel_sb[:, 128 * t:128 * (t + 1)]
                        st = t == 0
                        sp = t == NT - 1
                        nc.tensor.matmul(ps_S[:, 0:NB], lhs, zb0,
                                         start=st, stop=sp,
                                         tile_position=(0, 0),
                                         skip_group_check=True)
                        nc.tensor.matmul(ps_S[:, NB:COLS], lhs, zb1,
                                         start=st, stop=sp,
                                         tile_position=(0, 0),
                                         skip_group_check=True)
                        nc.tensor.matmul(ps_Q[:, 0:NB], lhs, zzb[:, 0:NB],
                                         start=st, stop=sp,
                                         tile_position=(0, 0),
                                         skip_group_check=True)
                        nc.tensor.matmul(ps_Q[:, NB:COLS], lhs, zzb[:, NB:COLS],
                                         start=st, stop=sp,
                                         tile_position=(0, 0),
                                         skip_group_check=True)
                        nc.tensor.matmul(ps_X[:, :], lhs, zxb[:],
                                         start=st, stop=sp,
                                         tile_position=(0, 0),
                                         skip_group_check=True)

                    # ---------- phase 2: 2x2 inverse sqrt + gamma fold ------
                    Q0, Q1 = ps_Q[:, 0:FP], ps_Q[:, FP:COLS]
                    X_ = ps_X[:, :]

                    P_ = ph2.tile([128, COLS], f32)    # (P0|P1)
                    P01 = ph2.tile([128, FP], f32)
                    C_ = ph2.tile([128, COLS], f32)    # (C0|C1) -> (w00|w11)
                    Xc = ph2.tile([128, FP], f32)
                    d_ = ph2.tile([128, FP], f32)
                    s_ = ph2.tile([128, FP], f32)
                    u_ = ph2.tile([128, FP], f32)
                    r_ = ph2.tile([128, FP], f32)
                    # f32 A/R planes: Af = (A00|A10|A01|A11), Rf = (R0|R1)
                    Af = ph2.tile([128, 4 * FP], f32)
                    Rf = ph2.tile([128, COLS], f32)
                    # scratch aliases on dead tiles
                    x2 = P01     # dead after Xc computed (reuse halves)
                    tq = d_      # dead after s_ = sqrt(d)
                    T_ = P_[:, 0:FP]   # P_ dead after C_ computed
                    ra = s_      # dead after w-planes are scaled
                    rb = u_      # dead after tq computed

                    Ssb = ph2.tile([128, COLS], f32)
                    nc.scalar.copy(Ssb[:], ps_S[:, :])
                    S0, S1 = Ssb[:, 0:FP], Ssb[:, FP:COLS]
                    nc.scalar.activation(P_[:], Ssb[:], AF.Square,
                                         scale=float(irB))
                    nc.vector.scalar_tensor_tensor(P01[:], S0, float(irB * irB),
                                                   S1, OP.mult, OP.mult)
                    nc.vector.tensor_tensor(C_[:], ps_Q[:, :], P_[:],
                                            OP.subtract)
                    C0, C1 = C_[:, 0:FP], C_[:, FP:COLS]
                    nc.vector.tensor_tensor(Xc[:], X_, P01[:], OP.subtract)
                    nc.gpsimd.tensor_tensor(d_[:], C0, C1, OP.mult)
                    nc.scalar.square(x2[:], Xc[:])
                    nc.vector.tensor_tensor(d_[:], d_[:], x2[:], OP.subtract)
                    nc.scalar.activation(s_[:], d_[:], AF.Sqrt)
                    nc.gpsimd.tensor_tensor(u_[:], C0, C1, OP.add)
                    nc.vector.scalar_tensor_tensor(u_[:], s_[:], 2.0, u_[:],
                                                   OP.mult, OP.add)
                    # r = 1/sqrt((B-1) u)
                    nc.scalar.activation(tq[:], u_[:], AF.Sqrt,
                                         scale=float(B - 1))
                    nc.vector.reciprocal(r_[:], tq[:])
                    # w00 = (C0 + s) r ; w11 = (C1 + s) r ; w01 = Xc r
                    nc.vector.tensor_tensor(C0, C0, s_[:], OP.add)
                    nc.gpsimd.tensor_tensor(C1, C1, s_[:], OP.add)
                    nc.vector.tensor_tensor(C0, C0, r_[:], OP.mult)
                    nc.gpsimd.tensor_tensor(C1, C1, r_[:], OP.mult)
                    nc.vector.tensor_tensor(Xc[:], Xc[:], r_[:], OP.mult)
                    w00, w11, w01 = C0, C1, Xc[:]

                    # A = gamma @ W (2x2), de-interleaved into Af
                    A00 = Af[:, 0:FP]
                    A10 = Af[:, FP:2 * FP]
                    A01 = Af[:, 2 * FP:3 * FP]
                    A11 = Af[:, 3 * FP:4 * FP]
                    nc.vector.tensor_scalar(T_, w00, g00c, None, OP.mult)
                    nc.vector.scalar_tensor_tensor(A00, w01, g01c, T_,
                                                   OP.mult, OP.add)
                    nc.vector.tensor_scalar(T_, w00, g10c, None, OP.mult)
                    nc.vector.scalar_tensor_tensor(A10, w01, g11c, T_,
                                                   OP.mult, OP.add)
                    nc.vector.tensor_scalar(T_, w11, g01c, None, OP.mult)
                    nc.vector.scalar_tensor_tensor(A01, w01, g00c, T_,
                                                   OP.mult, OP.add)
                    nc.vector.tensor_scalar(T_, w11, g11c, None, OP.mult)
                    nc.vector.scalar_tensor_tensor(A11, w01, g10c, T_,
                                                   OP.mult, OP.add)
                    # R_i = b_i - (Ai0 S0 + Ai1 S1)/B
                    # R_i = b_i + Ai0 (-S0/B) + Ai1 (-S1/B); pure TTs so the
                    # R1 chain can run on Pool (no PSUM / no tensor_scalar)
                    Ssc = ph2.tile([128, COLS], f32)
                    nc.scalar.mul(Ssc[:], Ssb[:], float(-1.0 / B))
                    Ssc0, Ssc1 = Ssc[:, 0:FP], Ssc[:, FP:COLS]
                    b0rep = bass.AP(tensor=gcols.tensor, offset=gcols.offset + 4,
                                    ap=[list(gcols.ap[0]), [0, FP]])
                    b1rep = bass.AP(tensor=gcols.tensor, offset=gcols.offset + 5,
                                    ap=[list(gcols.ap[0]), [0, FP]])
                    rc = Xc      # dead after A-coeffs
                    rd = d_      # dead after reciprocal
                    nc.vector.tensor_tensor(ra[:], A00, Ssc0, OP.mult)
                    nc.vector.tensor_tensor(rc[:], A01, Ssc1, OP.mult)
                    nc.vector.tensor_tensor(ra[:], ra[:], rc[:], OP.add)
                    nc.vector.tensor_tensor(Rf[:, 0:FP], ra[:], b0rep, OP.add)
                    nc.gpsimd.tensor_tensor(rb[:], A10, Ssc0, OP.mult)
                    nc.gpsimd.tensor_tensor(rd[:], A11, Ssc1, OP.mult)
                    nc.gpsimd.tensor_tensor(rb[:], rb[:], rd[:], OP.add)
                    nc.gpsimd.tensor_tensor(Rf[:, FP:COLS], rb[:], b1rep,
                                            OP.add)

                # bf16 coefficient planes
                Pap = ph2.tile([128, COLS], bf16, tag="Pap")
                Qap = ph2.tile([128, COLS], bf16, tag="Qap")
                Rap = ph2.tile([128, COLS], bf16, tag="Rap")
                nc.scalar.copy(Pap[:], Af[:, 0:COLS])
                nc.scalar.copy(Qap[:], Af[:, COLS:2 * COLS])
                nc.scalar.copy(Rap[:], Rf[:])

                # ---------- phase 3: broadcast + apply ----------
                with (
                    tc.tile_pool(name="psum3a", bufs=2, space="PSUM") as psum3a,
                    tc.tile_pool(name="psum3r", bufs=2, space="PSUM") as psum3r,
                ):
                    # software-pipelined, engine-visit-once-per-iteration:
                    #  iter k: PE bcasts(k); PE R+I-accum(k-1); ACT casts(k);
                    #          DVE muls(k); DVE re-interleave copy(k-2); DMA.
                    # The one-iteration lag on the I-accumulates keeps the
                    # in-order PE stream from serializing on DVE's muls.
                    cA = {}
                    cB = {}

                    def _stage_bcast(t):
                        g, i = divmod(t, 8)
                        rows = slice(32 * g, 32 * (g + 1))
                        lhs_b = ind_sb[rows, 128 * i:128 * (i + 1)]
                        ps_P = psum3a.tile([128, COLS], f32, tag="psA")
                        ps_Qb = psum3a.tile([128, COLS], f32, tag="psA")
                        ps_R = psum3r.tile([128, COLS], f32, tag="psR")
                        for h in range(2):
                            cs = slice(h * NB, (h + 1) * NB)
                            nc.tensor.matmul(ps_P[:, cs], lhs_b,
                                             Pap[rows, cs],
                                             start=True, stop=True,
                                             tile_position=(32 * g, 0),
                                             skip_group_check=True)
                            nc.tensor.matmul(ps_Qb[:, cs], lhs_b,
                                             Qap[rows, cs],
                                             start=True, stop=True,
                                             tile_position=(32 * g, 0),
                                             skip_group_check=True)
                            nc.tensor.matmul(ps_R[:, cs], lhs_b,
                                             Rap[rows, cs],
                                             start=True, stop=True,
                                             tile_position=(32 * g, 0),
                                             skip_group_check=True)
                        zoff = t * COLS
                        z0rep = bass.AP(tensor=zb_all.tensor, offset=zoff,
                                        ap=[list(zb_all.ap[0]), [0, 2],
                                            [1, FP]])
                        z1rep = bass.AP(tensor=zb_all.tensor, offset=zoff + FP,
                                        ap=[list(zb_all.ap[0]), [0, 2],
                                            [1, FP]])
                        ta = work2.tile([128, COLS], bf16, tag="ta", bufs=4)
                        tb = work2.tile([128, COLS], bf16, tag="tb", bufs=4)
                        ta_v = bass.AP(tensor=ta.tensor, offset=ta.offset,
                                       ap=[list(ta.ap[0]), [FP, 2], [1, FP]])
                        tb_v = bass.AP(tensor=tb.tensor, offset=tb.offset,
                                       ap=[list(tb.ap[0]), [FP, 2], [1, FP]])
                        psP_v = bass.AP(tensor=ps_P.tensor, offset=ps_P.offset,
                                        ap=[list(ps_P.ap[0]), [FP, 2],
                                            [1, FP]])
                        psQ_v = bass.AP(tensor=ps_Qb.tensor,
                                        offset=ps_Qb.offset,
                                        ap=[list(ps_Qb.ap[0]), [FP, 2],
                                            [1, FP]])
                        nc.vector.tensor_tensor(ta_v, psP_v, z0rep, OP.mult)
                        nc.vector.tensor_tensor(tb_v, psQ_v, z1rep, OP.mult)
                        tt = work2.tile([128, COLS], bf16, tag="tt", bufs=4)
                        nc.vector.tensor_tensor(tt[:], ta[:], tb[:], OP.add)
                        # ob[2n+c] = tt[c*FP+n] + ps_R[c*FP+n] (re-interleave)
                        ob = obpool.tile([128, COLS], f32, tag="ob")
                        ob_v = bass.AP(tensor=ob.tensor, offset=ob.offset,
                                       ap=[list(ob.ap[0]), [1, 2], [2, FP]])
                        tt_v = bass.AP(tensor=tt.tensor, offset=tt.offset,
                                       ap=[list(tt.ap[0]), [FP, 2], [1, FP]])
                        psR_v = bass.AP(tensor=ps_R.tensor, offset=ps_R.offset,
                                        ap=[list(ps_R.ap[0]), [FP, 2],
                                            [1, FP]])
                        nc.vector.tensor_tensor(ob_v, tt_v, psR_v, OP.add)
                        nc.sync.dma_start(out=out_r[t], in_=ob[:])

                    for it in range(NT):
                        _stage_bcast(it)

            for _rep in range(reps):
                _pipeline()

    nc.compile()
    return nc


_NC = {}


def _get_module(reps=1):
    if reps not in _NC:
        _NC[reps] = build_module(reps)
    return _NC[reps]


def _in_maps_for(z, gamma, beta):
    z = np.ascontiguousarray(z, dtype=np.float32)
    gamma = np.ascontiguousarray(gamma, dtype=np.float32)
    beta = np.ascontiguousarray(beta, dtype=np.float32)
    zr = z.reshape(B, C, H * W * 2)
    sel32, ind = _host_constants()
    ident = np.eye(128, dtype=np.float32).astype(ml_dtypes.bfloat16)
    in_maps = []
    for c in range(NCORES):
        shard = np.ascontiguousarray(
            zr[:, c * C_PER:(c + 1) * C_PER].reshape(B, M))
        in_maps.append({"z": shard, "gamma": gamma, "beta": beta,
                        "sel32": sel32, "ind": ind, "ident": ident})
    return in_maps


def kernel(z, gamma, beta):
    in_maps = _in_maps_for(z, gamma, beta)
    m1 = _get_module(1)
    runner = _get_runner(("m", id(m1)), m1, NCORES)
    results = _run_module(runner, in_maps)
    out = np.empty((B, C, H * W * 2), dtype=np.float32)
    for c in range(NCORES):
        out[:, c * C_PER:(c + 1) * C_PER] = results[c]["out"].reshape(
            B, C_PER, H * W * 2)
    return out.reshape(B, C, H, W, 2)


def _build_memcpy_module(reps=1):
    """Baseline: per-core DMA z -> out through SBUF (same traffic as kernel)."""
    nc = bacc.Bacc("TRN2", target_bir_lowering=False, debug=False,
                   detect_race_conditions=False)
    z_d = nc.dram_tensor("z", [B, M], f32, kind="ExternalInput").ap()
    out_d = nc.dram_tensor("out", [B, M], f32, kind="ExternalOutput").ap()
    z_r = z_d.rearrange("b (t j f) -> t j b f", t=NT, j=J, f=COLS)
    out_r = out_d.rearrange("b (t j f) -> t j b f", t=NT, j=J, f=COLS)
    with tile.TileContext(nc) as tc:
        with tc.tile_pool(name="buf", bufs=6) as buf:
            for _ in range(reps):
                for t in range(NT):
                    x = buf.tile([128, COLS], f32, tag="x")
                    nc.sync.dma_start(out=x[:], in_=z_r[t])
                    nc.scalar.dma_start(out=out_r[t], in_=x[:])
    nc.compile()
    return nc


def bench_memcpy(z, iters=10, reps=17):
    z = np.ascontiguousarray(z, dtype=np.float32)
    zr = z.reshape(B, C, H * W * 2)
    in_maps = []
    for c in range(NCORES):
        shard = np.ascontiguousarray(
            zr[:, c * C_PER:(c + 1) * C_PER].reshape(B, M))
        in_maps.append({"z": shard})
    ta, tb = bench_pair((_build_memcpy_module(1), _build_memcpy_module(reps)),
                        in_maps, in_maps, iters=iters, rounds=4)
    slopes = sorted((b - a) / (reps - 1) for a, b in zip(ta, tb))
    return slopes[len(slopes) // 2]


def _make_runner(nc, n_cores):
    """Build (and cache) the sharded jit executable for an SPMD module."""
    import jax
    import jax.numpy as jnp
    from jax.sharding import Mesh, PartitionSpec
    from jax.experimental.shard_map import shard_map
    from concourse import bass2jax
    from concourse.bass2jax import _bass_exec_p, install_neuronx_cc_hook
    from concourse import mybir as _mb

    install_neuronx_cc_hook()
    partition_name = (nc.partition_id_tensor.name
                      if nc.partition_id_tensor else None)
    in_names, out_names, out_avals, zero_outs = [], [], [], []
    for alloc in nc.m.functions[0].allocations:
        if not isinstance(alloc, _mb.MemoryLocationSet):
            continue
        name = alloc.memorylocations[0].name
        if alloc.kind == "ExternalInput":
            if name != partition_name:
                in_names.append(name)
        elif alloc.kind == "ExternalOutput":
            shape = tuple(alloc.tensor_shape)
            dtype = _mb.dt.np(alloc.dtype)
            out_names.append(name)
            out_avals.append(jax.core.ShapedArray(shape, dtype))
            zero_outs.append(np.zeros(shape, dtype))
    n_params = len(in_names)
    n_outs = len(out_avals)
    all_in_names = in_names + out_names
    if partition_name is not None:
        all_in_names.append(partition_name)

    def _body(*args):
        operands = list(args)
        if partition_name is not None:
            operands.append(bass2jax.partition_id_tensor())
        outs = _bass_exec_p.bind(
            *operands,
            out_avals=tuple(out_avals),
            in_names=tuple(all_in_names),
            out_names=tuple(out_names),
            lowering_input_output_aliases=(),
            sim_require_finite=True,
            sim_require_nnan=True,
            nc=nc,
        )
        return tuple(outs)

    devices = jax.devices()[:n_cores]
    mesh = Mesh(np.asarray(devices), ("core",))
    donate = tuple(range(n_params, n_params + n_outs))
    sharded = jax.jit(
        shard_map(_body, mesh=mesh,
                  in_specs=(PartitionSpec("core"),) * (n_params + n_outs),
                  out_specs=(PartitionSpec("core"),) * n_outs,
                  check_rep=False),
        donate_argnums=donate, keep_unused=True,
    )
    from jax.sharding import NamedSharding
    shard0 = NamedSharding(mesh, PartitionSpec("core"))
    return {
        "sharded": sharded, "shard0": shard0, "in_names": in_names,
        "out_names": out_names, "out_avals": out_avals,
        "zero_outs": zero_outs, "n_cores": n_cores,
    }


_RUNNERS = {}


def _get_runner(key, nc, n_cores):
    if key not in _RUNNERS:
        _RUNNERS[key] = _make_runner(nc, n_cores)
    return _RUNNERS[key]


def _run_module(runner, in_maps):
    import jax
    n_cores = runner["n_cores"]
    concat_in = [
        jax.device_put(
            np.concatenate([np.asarray(m[name]) for m in in_maps], axis=0),
            runner["shard0"])
        for name in runner["in_names"]
    ]
    zeros = [
        jax.device_put(
            np.zeros((n_cores * z.shape[0], *z.shape[1:]), z.dtype),
            runner["shard0"])
        for z in runner["zero_outs"]
    ]
    outs = runner["sharded"](*concat_in, *zeros)
    jax.block_until_ready(outs)
    return [
        {name: np.asarray(outs[i]).reshape(
            n_cores, *runner["out_avals"][i].shape)[c]
         for i, name in enumerate(runner["out_names"])}
        for c in range(n_cores)
    ]


def bench_module(nc, in_maps, iters=12, key=None):
    """Min-of-per-call timing of an SPMD bass module via the PJRT path."""
    import time as _time
    import jax
    runner = _make_runner(nc, len(in_maps))
    n_cores = runner["n_cores"]
    shard0 = runner["shard0"]
    sharded = runner["sharded"]
    concat_in = [
        jax.device_put(
            np.concatenate([np.asarray(m[name]) for m in in_maps], axis=0),
            shard0)
        for name in runner["in_names"]
    ]
    zero_sets = []
    for _ in range(iters + 1):
        zero_sets.append([
            jax.device_put(
                np.zeros((n_cores * z.shape[0], *z.shape[1:]), z.dtype),
                shard0)
            for z in runner["zero_outs"]
        ])
    outs = sharded(*concat_in, *zero_sets[0])
    jax.block_until_ready(outs)

    def one_batch(ks):
        t0 = _time.perf_counter()
        last = None
        for k in ks:
            last = sharded(*concat_in, *zero_sets[k + 1])
        jax.block_until_ready(last)
        return (_time.perf_counter() - t0) / len(ks), last

    dt, last = one_batch(range(iters))
    results = [
        {name: np.asarray(last[i]).reshape(
            n_cores, *runner["out_avals"][i].shape)[c]
         for i, name in enumerate(runner["out_names"])}
        for c in range(n_cores)
    ]
    return dt * 1e9, results


def bench_pair(ncs, in_maps_a, in_maps_b, iters=8, rounds=4):
    """Interleaved async-batch timing of two modules; returns
    (median per-call ns A, median per-call ns B, per-round lists)."""
    import time as _time
    import jax
    runners = [_get_runner(("m", id(ncs[0])), ncs[0], len(in_maps_a)),
               _get_runner(("m", id(ncs[1])), ncs[1], len(in_maps_b))]
    sides = []
    for runner, im in ((runners[0], in_maps_a), (runners[1], in_maps_b)):
        concat_in = [
            jax.device_put(
                np.concatenate([np.asarray(m[name]) for m in im], axis=0),
                runner["shard0"])
            for name in runner["in_names"]
        ]
        n_cores = runner["n_cores"]
        zsets = []
        for _ in range(iters * rounds + 1):
            zsets.append([
                jax.device_put(
                    np.zeros((n_cores * z.shape[0], *z.shape[1:]), z.dtype),
                    runner["shard0"])
                for z in runner["zero_outs"]
            ])
        sides.append((runner, concat_in, zsets))
        out = runner["sharded"](*concat_in, *zsets[0])
        jax.block_until_ready(out)
    ta, tb = [], []
    k = [0, 0]
    for r in range(rounds):
        for side, rec in ((0, ta), (1, tb)):
            runner, concat_in, zsets = sides[side]
            t0 = _time.perf_counter()
            last = None
            for _ in range(iters):
                k[side] += 1
                last = runner["sharded"](*concat_in, *zsets[k[side]])
            jax.block_until_ready(last)
            rec.append((_time.perf_counter() - t0) / iters * 1e9)
    return ta, tb


def bench(z, gamma, beta, iters=10, reps=17, with_memcpy=False):
    """Slope-based device timing: time modules with `reps`=1 and `reps`=R
    internal repetitions of the full pipeline; per-kernel device time =
    (t_R - t_1) / (R - 1), which cancels the per-dispatch axon overhead."""
    in_maps = _in_maps_for(z, gamma, beta)
    ta, tb = bench_pair((_get_module(1), _get_module(reps)),
                        in_maps, in_maps, iters=iters, rounds=4)
    slopes = sorted((b - a) / (reps - 1) for a, b in zip(ta, tb))
    ns = slopes[len(slopes) // 2]
    m1 = _get_module(1)
    runner = _get_runner(("m", id(m1)), m1, NCORES)
    results = _run_module(runner, in_maps)
    t1_ns, tR_ns = min(ta), min(tb)
    out = np.empty((B, C, H * W * 2), dtype=np.float32)
    for c in range(NCORES):
        out[:, c * C_PER:(c + 1) * C_PER] = results[c]["out"].reshape(
            B, C_PER, H * W * 2)
    return out.reshape(B, C, H, W, 2), ns, (t1_ns, tR_ns)


def run_traced(z, gamma, beta):
    """Like kernel() but with NTFF tracing; returns (output, exec_time_ns)."""
    in_maps = _in_maps_for(z, gamma, beta)
    nc = _get_module()
    res = run_bass_kernel_spmd(nc, in_maps, core_ids=list(range(NCORES)),
                               trace=True)
    out = np.empty((B, C, H * W * 2), dtype=np.float32)
    for c in range(NCORES):
        out[:, c * C_PER:(c + 1) * C_PER] = res.results[c]["out"].reshape(
            B, C_PER, H * W * 2)
    return out.reshape(B, C, H, W, 2), res.exec_time_ns, res


# revision 31
# speedup vs baseline: 1.0406x; 1.0406x over previous
"""Trainium2 Bass kernel for nn_C_BatchNorm (complex batch-norm, training mode).

Problem: z [B=32, C=128, H=64, W=64, 2] fp32.  Per position n=(c,h,w):
  2x2 covariance over batch, closed-form inverse sqrt, whiten, gamma/beta.

Sharding: C split across 8 cores (16 channels each).  Per core the shard is
[32, 131072] fp32 (16 MiB in / 16 MiB out), processed as 32 tiles of
[128 partitions = 4 position-groups x 32 batch, 1024 = 512 positions x 2
interleaved components].

v2 design (bf16 resident, de-interleaved):
  Phase 1 (per tile): DMA f32 tile -> staging; ACT copy de-interleaves and
    casts to bf16 resident zb (z0|z1 halves); DVE computes zz=zb*zb and
    zx=zb0*zb1 in bf16 2x mode; 5 bf16 matmuls with per-tile [128,128]
    selectors accumulate S|Q|X moments for ALL tiles directly into PSUM
    rows 32*(t//8) + 4*(t%8) + j -- no staging, no repack.
  Phase 2 (once): closed-form 2x2 inverse-sqrt + gamma fold on de-interleaved
    [128,512] planes straight from PSUM -> bf16 coefficient planes
    Pap=(A00|A10), Qap=(A01|A11), Rap=(R0|R1) where out_i = Ai0 z0 + Ai1 z1
    + Ri.
  Phase 3 (per tile): 6 bf16 indicator matmuls broadcast the tile's 4
    coefficient rows to 128 partitions in PSUM; ACT casts P/Q broadcast to
    bf16 SBUF; DVE (2x mode) multiplies with rep-views of zb and adds;
    Pool adds the R broadcast (PSUM) and re-interleaves into the f32 output
    tile, which is DMA'd out.
"""

import numpy as np
import ml_dtypes

import concourse.bass as bass
import concourse.bacc as bacc
import concourse.tile as tile
from concourse import mybir
from concourse.bass_utils import run_bass_kernel_spmd

f32 = mybir.dt.float32
bf16 = mybir.dt.bfloat16
f32r = mybir.dt.float32r
AF = mybir.ActivationFunctionType
OP = mybir.AluOpType

# ---- problem geometry (hardcoded) ----
B, C, H, W = 32, 128, 64, 64
NCORES = 8
C_PER = C // NCORES                  # 16 channels per core
NPOS = C_PER * H * W                 # 65536 positions per core
M = NPOS * 2                         # 131072 fp32 per batch row per core
NT = 32                              # tiles per core
FP = 512                             # positions per group per tile
COLS = 2 * FP                        # 1024 fp32 per partition per tile
J = 4                                # position groups per tile (32 batch each)
NB = 512                             # matmul free-dim chunk (one PSUM bank)


def _host_constants():
    # sel32[t]: [128, 128] with sel[p, w] = 1 iff w == 32*(t//8)+4*(t%8)+p//32
    sel = np.zeros((128, NT, 128), dtype=np.float32)
    for t in range(NT):
        g, i = divmod(t, 8)
        for p in range(128):
            sel[p, t, 32 * g + 4 * i + p // 32] = 1.0
    sel = sel.reshape(128, NT * 128).astype(ml_dtypes.bfloat16)
    # ind for phase-3 broadcast: 8 variants [128, 128] (row pattern mod 32):
    # ind[p, 128*i + q] = 1 iff (p % 32) == 4*i + q//32
    ind = np.zeros((128, 8, 128), dtype=np.float32)
    for i in range(8):
        for p in range(128):
            for jj in range(4):
                if p % 32 == 4 * i + jj:
                    ind[p, i, 32 * jj:32 * (jj + 1)] = 1.0
    ind = ind.reshape(128, 8 * 128).astype(ml_dtypes.bfloat16)
    return sel, ind


def build_module(reps=1):
    nc = bacc.Bacc("TRN2", target_bir_lowering=False, debug=False,
                   detect_race_conditions=False)
    z_d = nc.dram_tensor("z", [B, M], f32, kind="ExternalInput").ap()
    gamma_d = nc.dram_tensor("gamma", [2, 2], f32, kind="ExternalInput").ap()
    beta_d = nc.dram_tensor("beta", [2], f32, kind="ExternalInput").ap()
    sel_d = nc.dram_tensor("sel32", [128, NT * 128], bf16,
                           kind="ExternalInput").ap()
    ind_d = nc.dram_tensor("ind", [128, 8 * 128], bf16,
                           kind="ExternalInput").ap()
    ident_d = nc.dram_tensor("ident", [128, 128], bf16,
                             kind="ExternalInput").ap()
    out_d = nc.dram_tensor("out", [B, M], f32, kind="ExternalOutput").ap()

    # DRAM views ordered [tile, group, batch, col] (partition p = 32*j + b)
    z_r = z_d.rearrange("b (t j f) -> t j b f", t=NT, j=J, f=COLS)
    out_r = out_d.rearrange("b (t j f) -> t j b f", t=NT, j=J, f=COLS)

    irB = 1.0 / np.sqrt(np.float32(B))      # 1/sqrt(B)

    with tile.TileContext(nc) as tc:
        with (
            tc.tile_pool(name="consts", bufs=1) as consts,
            tc.tile_pool(name="zres", bufs=1) as zres,
            tc.tile_pool(name="ph2", bufs=1) as ph2,
            tc.tile_pool(name="work", bufs=3) as work,
            tc.tile_pool(name="work2", bufs=3) as work2,
            tc.tile_pool(name="obpool", bufs=3) as obpool,
        ):
            # ---------- constants ----------
            sel_sb = consts.tile([128, NT * 128], bf16)
            nc.sync.dma_start(out=sel_sb[:], in_=sel_d)
            ind_sb = consts.tile([128, 8 * 128], bf16)
            nc.sync.dma_start(out=ind_sb[:], in_=ind_d)
            ident_sb = consts.tile([128, 128], bf16)
            nc.sync.dma_start(out=ident_sb[:], in_=ident_d)

            gcols = consts.tile([128, 6], f32)   # g00 g01 g10 g11 b0 b1
            for k in range(4):
                nc.gpsimd.dma_start(
                    out=gcols[:, k:k + 1],
                    in_=bass.AP(tensor=gamma_d.tensor, offset=k,
                                ap=[[0, 128], [1, 1]]))
            for k in range(2):
                nc.gpsimd.dma_start(
                    out=gcols[:, 4 + k:5 + k],
                    in_=bass.AP(tensor=beta_d.tensor, offset=k,
                                ap=[[0, 128], [1, 1]]))
            # preload ACT spline tables (Square/Sqrt) during phase 0
            warm = consts.tile([128, 2], f32)
            nc.scalar.square(warm[:, 0:1], gcols[:, 0:1])
            nc.scalar.activation(warm[:, 1:2], warm[:, 0:1], AF.Sqrt)
            g00c, g01c = gcols[:, 0:1], gcols[:, 1:2]
            g10c, g11c = gcols[:, 2:3], gcols[:, 3:4]
            b0c, b1c = gcols[:, 4:5], gcols[:, 5:6]

            # resident z for the whole core, bf16, de-interleaved per tile:
            # tile t occupies cols [t*COLS, t*COLS+FP) = z0, [+FP, +COLS) = z1
            zb_all = zres.tile([128, NT * COLS], bf16)

            def _pipeline():
                tc.tile_update_base_wait()
                # ---------- phase 1: moments straight into PSUM ----------
                with tc.tile_pool(name="psum1", bufs=1, space="PSUM") as psum1:
                    ps_S = psum1.tile([128, COLS], f32)
                    ps_Q = psum1.tile([128, COLS], f32)
                    ps_X = psum1.tile([128, FP], f32)
                    for t in range(NT):
                        zf = work.tile([128, COLS], f32, tag="zf")
                        nc.sync.dma_start(out=zf[:], in_=z_r[t])
                        zb = zb_all[:, t * COLS:(t + 1) * COLS]
                        # de-interleave + cast: zb[c*FP + n] = zf[2n + c]
                        zb_v = bass.AP(tensor=zb_all.tensor,
                                       offset=t * COLS,
                                       ap=[list(zb_all.ap[0]), [FP, 2], [1, FP]])
                        zf_v = bass.AP(tensor=zf.tensor, offset=zf.offset,
                                       ap=[list(zf.ap[0]), [1, 2], [2, FP]])
                        nc.scalar.copy(zb_v, zf_v)
                        zb0 = zb_all[:, t * COLS:t * COLS + FP]
                        zb1 = zb_all[:, t * COLS + FP:(t + 1) * COLS]
                        zzb = work.tile([128, COLS], bf16, tag="zz")
                        nc.vector.tensor_tensor(zzb[:], zb, zb, OP.mult)
                        zxb = work.tile([128, FP], bf16, tag="zx")
                        nc.vector.tensor_tensor(zxb[:], zb0, zb1, OP.mult)

                        lhs = sel_sb[:, 128 * t:128 * (t + 1)]
                        st = t == 0
                        sp = t == NT - 1
                        nc.tensor.matmul(ps_S[:, 0:NB], lhs, zb0,
                                         start=st, stop=sp,
                                         tile_position=(0, 0),
                                         skip_group_check=True)
                        nc.tensor.matmul(ps_S[:, NB:COLS], lhs, zb1,
                                         start=st, stop=sp,
                                         tile_position=(0, 0),
                                         skip_group_check=True)
                        nc.tensor.matmul(ps_Q[:, 0:NB], lhs, zzb[:, 0:NB],
                                         start=st, stop=sp,
                                         tile_position=(0, 0),
                                         skip_group_check=True)
                        nc.tensor.matmul(ps_Q[:, NB:COLS], lhs, zzb[:, NB:COLS],
                                         start=st, stop=sp,
                                         tile_position=(0, 0),
                                         skip_group_check=True)
                        nc.tensor.matmul(ps_X[:, :], lhs, zxb[:],
                                         start=st, stop=sp,
                                         tile_position=(0, 0),
                                         skip_group_check=True)

                    # ---------- phase 2: 2x2 inverse sqrt + gamma fold ------
                    Q0, Q1 = ps_Q[:, 0:FP], ps_Q[:, FP:COLS]
                    X_ = ps_X[:, :]

                    P_ = ph2.tile([128, COLS], f32)    # (P0|P1)
                    P01 = ph2.tile([128, FP], f32)
                    C_ = ph2.tile([128, COLS], f32)    # (C0|C1) -> (w00|w11)
                    Xc = ph2.tile([128, FP], f32)
                    d_ = ph2.tile([128, FP], f32)
                    s_ = ph2.tile([128, FP], f32)
                    u_ = ph2.tile([128, FP], f32)
                    r_ = ph2.tile([128, FP], f32)
                    # f32 A/R planes: Af = (A00|A10|A01|A11), Rf = (R0|R1)
                    Af = ph2.tile([128, 4 * FP], f32)
                    Rf = ph2.tile([128, COLS], f32)
                    # scratch aliases on dead tiles
                    x2 = P01     # dead after Xc computed (reuse halves)
                    tq = d_      # dead after s_ = sqrt(d)
                    T_ = P_[:, 0:FP]   # P_ dead after C_ computed
                    ra = s_      # dead after w-planes are scaled
                    rb = u_      # dead after tq computed

                    Ssb = ph2.tile([128, COLS], f32)
                    nc.scalar.copy(Ssb[:], ps_S[:, :])
                    S0, S1 = Ssb[:, 0:FP], Ssb[:, FP:COLS]
                    nc.scalar.activation(P_[:], Ssb[:], AF.Square,
                                         scale=float(irB))
                    nc.vector.scalar_tensor_tensor(P01[:], S0, float(irB * irB),
                                                   S1, OP.mult, OP.mult)
                    nc.vector.tensor_tensor(C_[:], ps_Q[:, :], P_[:],
                                            OP.subtract)
                    C0, C1 = C_[:, 0:FP], C_[:, FP:COLS]
                    nc.vector.tensor_tensor(Xc[:], X_, P01[:], OP.subtract)
                    nc.gpsimd.tensor_tensor(d_[:], C0, C1, OP.mult)
                    nc.scalar.square(x2[:], Xc[:])
                    nc.vector.tensor_tensor(d_[:], d_[:], x2[:], OP.subtract)
                    nc.scalar.activation(s_[:], d_[:], AF.Sqrt)
                    nc.gpsimd.tensor_tensor(u_[:], C0, C1, OP.add)
                    nc.vector.scalar_tensor_tensor(u_[:], s_[:], 2.0, u_[:],
                                                   OP.mult, OP.add)
                    # r = 1/sqrt((B-1) u)
                    nc.scalar.activation(tq[:], u_[:], AF.Sqrt,
                                         scale=float(B - 1))
                    nc.vector.reciprocal(r_[:], tq[:])
                    # w00 = (C0 + s) r ; w11 = (C1 + s) r ; w01 = Xc r
                    nc.vector.tensor_tensor(C0, C0, s_[:], OP.add)
                    nc.gpsimd.tensor_tensor(C1, C1, s_[:], OP.add)
                    nc.vector.tensor_tensor(C0, C0, r_[:], OP.mult)
                    nc.gpsimd.tensor_tensor(C1, C1, r_[:], OP.mult)
                    nc.vector.tensor_tensor(Xc[:], Xc[:], r_[:], OP.mult)
                    w00, w11, w01 = C0, C1, Xc[:]

                    # A = gamma @ W (2x2), de-interleaved into Af
                    A00 = Af[:, 0:FP]
                    A10 = Af[:, FP:2 * FP]
                    A01 = Af[:, 2 * FP:3 * FP]
                    A11 = Af[:, 3 * FP:4 * FP]
                    nc.vector.tensor_scalar(T_, w00, g00c, None, OP.mult)
                    nc.vector.scalar_tensor_tensor(A00, w01, g01c, T_,
                                                   OP.mult, OP.add)
                    nc.vector.tensor_scalar(T_, w00, g10c, None, OP.mult)
                    nc.vector.scalar_tensor_tensor(A10, w01, g11c, T_,
                                                   OP.mult, OP.add)
                    nc.vector.tensor_scalar(T_, w11, g01c, None, OP.mult)
                    nc.vector.scalar_tensor_tensor(A01, w01, g00c, T_,
                                                   OP.mult, OP.add)
                    nc.vector.tensor_scalar(T_, w11, g11c, None, OP.mult)
                    nc.vector.scalar_tensor_tensor(A11, w01, g10c, T_,
                                                   OP.mult, OP.add)
                    # R_i = b_i - (Ai0 S0 + Ai1 S1)/B
                    # R_i = b_i + Ai0 (-S0/B) + Ai1 (-S1/B); pure TTs so the
                    # R1 chain can run on Pool (no PSUM / no tensor_scalar)
                    Ssc = ph2.tile([128, COLS], f32)
                    nc.scalar.mul(Ssc[:], Ssb[:], float(-1.0 / B))
                    Ssc0, Ssc1 = Ssc[:, 0:FP], Ssc[:, FP:COLS]
                    b0rep = bass.AP(tensor=gcols.tensor, offset=gcols.offset + 4,
                                    ap=[list(gcols.ap[0]), [0, FP]])
                    b1rep = bass.AP(tensor=gcols.tensor, offset=gcols.offset + 5,
                                    ap=[list(gcols.ap[0]), [0, FP]])
                    rc = Xc      # dead after A-coeffs
                    rd = d_      # dead after reciprocal
                    nc.vector.tensor_tensor(ra[:], A00, Ssc0, OP.mult)
                    nc.vector.tensor_tensor(rc[:], A01, Ssc1, OP.mult)
                    nc.vector.tensor_tensor(ra[:], ra[:], rc[:], OP.add)
                    nc.vector.tensor_tensor(Rf[:, 0:FP], ra[:], b0rep, OP.add)
                    nc.gpsimd.tensor_tensor(rb[:], A10, Ssc0, OP.mult)
                    nc.gpsimd.tensor_tensor(rd[:], A11, Ssc1, OP.mult)
                    nc.gpsimd.tensor_tensor(rb[:], rb[:], rd[:], OP.add)
                    nc.gpsimd.tensor_tensor(Rf[:, FP:COLS], rb[:], b1rep,
                                            OP.add)

                # bf16 coefficient planes
                Pap = ph2.tile([128, COLS], bf16, tag="Pap")
                Qap = ph2.tile([128, COLS], bf16, tag="Qap")
                Rap = ph2.tile([128, COLS], bf16, tag="Rap")
                nc.scalar.copy(Pap[:], Af[:, 0:COLS])
                nc.scalar.copy(Qap[:], Af[:, COLS:2 * COLS])
                nc.scalar.copy(Rap[:], Rf[:])

                # ---------- phase 3: broadcast + apply ----------
                with (
                    tc.tile_pool(name="psum3a", bufs=2, space="PSUM") as psum3a,
                    tc.tile_pool(name="psum3r", bufs=2, space="PSUM") as psum3r,
                ):
                    # software-pipelined, engine-visit-once-per-iteration:
                    #  iter k: PE bcasts(k); PE R+I-accum(k-1); ACT casts(k);
                    #          DVE muls(k); DVE re-interleave copy(k-2); DMA.
                    # The one-iteration lag on the I-accumulates keeps the
                    # in-order PE stream from serializing on DVE's muls.
                    cA = {}
                    cB = {}

                    def _stage_bcast(t):
                        g, i = divmod(t, 8)
                        rows = slice(32 * g, 32 * (g + 1))
                        lhs_b = ind_sb[rows, 128 * i:128 * (i + 1)]
                        ps_P = psum3a.tile([128, COLS], f32, tag="psA")
                        ps_Qb = psum3a.tile([128, COLS], f32, tag="psA")
                        for h in range(2):
                            cs = slice(h * NB, (h + 1) * NB)
                            nc.tensor.matmul(ps_P[:, cs], lhs_b,
                                             Pap[rows, cs],
                                             start=True, stop=True,
                                             tile_position=(32 * g, 0),
                                             skip_group_check=True)
                            nc.tensor.matmul(ps_Qb[:, cs], lhs_b,
                                             Qap[rows, cs],
                                             start=True, stop=True,
                                             tile_position=(32 * g, 0),
                                             skip_group_check=True)
                        zoff = t * COLS
                        z0rep = bass.AP(tensor=zb_all.tensor, offset=zoff,
                                        ap=[list(zb_all.ap[0]), [0, 2],
                                            [1, FP]])
                        z1rep = bass.AP(tensor=zb_all.tensor, offset=zoff + FP,
                                        ap=[list(zb_all.ap[0]), [0, 2],
                                            [1, FP]])
                        ta = work2.tile([128, COLS], bf16, tag="ta", bufs=4)
                        tb = work2.tile([128, COLS], bf16, tag="tb", bufs=4)
                        ta_v = bass.AP(tensor=ta.tensor, offset=ta.offset,
                                       ap=[list(ta.ap[0]), [FP, 2], [1, FP]])
                        tb_v = bass.AP(tensor=tb.tensor, offset=tb.offset,
                                       ap=[list(tb.ap[0]), [FP, 2], [1, FP]])
                        psP_v = bass.AP(tensor=ps_P.tensor, offset=ps_P.offset,
                                        ap=[list(ps_P.ap[0]), [FP, 2],
                                            [1, FP]])
                        psQ_v = bass.AP(tensor=ps_Qb.tensor,
                                        offset=ps_Qb.offset,
                                        ap=[list(ps_Qb.ap[0]), [FP, 2],
                                            [1, FP]])
                        nc.vector.tensor_tensor(ta_v, psP_v, z0rep, OP.mult)
                        nc.vector.tensor_tensor(tb_v, psQ_v, z1rep, OP.mult)
                        return ta, tb

                    def _stage_iacc(t, ta, tb):
                        g, i = divmod(t, 8)
                        rows = slice(32 * g, 32 * (g + 1))
                        lhs_b = ind_sb[rows, 128 * i:128 * (i + 1)]
                        ps_R = psum3r.tile([128, COLS], f32, tag="psR")
                        for h in range(2):
                            cs = slice(h * NB, (h + 1) * NB)
                            nc.tensor.matmul(ps_R[:, cs], lhs_b,
                                             Rap[rows, cs],
                                             start=True, stop=False,
                                             tile_position=(32 * g, 0),
                                             skip_group_check=True)
                            nc.tensor.matmul(ps_R[:, cs], ident_sb[:],
                                             ta[:, cs], start=False,
                                             stop=False,
                                             tile_position=(0, 0),
                                             skip_group_check=True)
                            nc.tensor.matmul(ps_R[:, cs], ident_sb[:],
                                             tb[:, cs], start=False,
                                             stop=True,
                                             tile_position=(0, 0),
                                             skip_group_check=True)
                        return (ps_R,)

                    def _stage_b(t, ps_R):
                        # ob[2n+c] = ps_R[c*FP+n]  (re-interleave copy)
                        ob = obpool.tile([128, COLS], f32, tag="ob")
                        ob_v = bass.AP(tensor=ob.tensor, offset=ob.offset,
                                       ap=[list(ob.ap[0]), [1, 2], [2, FP]])
                        psR_v = bass.AP(tensor=ps_R.tensor,
                                        offset=ps_R.offset,
                                        ap=[list(ps_R.ap[0]), [FP, 2],
                                            [1, FP]])
                        nc.scalar.copy(ob_v, psR_v)
                        nc.sync.dma_start(out=out_r[t], in_=ob[:])

                    for it in range(NT + 3):
                        with tc.tile_wait_until(0.068 + 0.0025 * it):
                            if it < NT:
                                cA[it] = _stage_bcast(it)
                            if 2 <= it <= NT + 1:
                                cB[it - 2] = _stage_iacc(it - 2,
                                                         *cA.pop(it - 2))
                            if it >= 3:
                                _stage_b(it - 3, *cB.pop(it - 3))

            for _rep in range(reps):
                _pipeline()

    nc.compile()
    return nc


_NC = {}


def _get_module(reps=1):
    if reps not in _NC:
        _NC[reps] = build_module(reps)
    return _NC[reps]


def _in_maps_for(z, gamma, beta):
    z = np.ascontiguousarray(z, dtype=np.float32)
    gamma = np.ascontiguousarray(gamma, dtype=np.float32)
    beta = np.ascontiguousarray(beta, dtype=np.float32)
    zr = z.reshape(B, C, H * W * 2)
    sel32, ind = _host_constants()
    ident = np.eye(128, dtype=np.float32).astype(ml_dtypes.bfloat16)
    in_maps = []
    for c in range(NCORES):
        shard = np.ascontiguousarray(
            zr[:, c * C_PER:(c + 1) * C_PER].reshape(B, M))
        in_maps.append({"z": shard, "gamma": gamma, "beta": beta,
                        "sel32": sel32, "ind": ind, "ident": ident})
    return in_maps


def kernel(z, gamma, beta):
    in_maps = _in_maps_for(z, gamma, beta)
    m1 = _get_module(1)
    runner = _get_runner(("m", id(m1)), m1, NCORES)
    results = _run_module(runner, in_maps)
    out = np.empty((B, C, H * W * 2), dtype=np.float32)
    for c in range(NCORES):
        out[:, c * C_PER:(c + 1) * C_PER] = results[c]["out"].reshape(
            B, C_PER, H * W * 2)
    return out.reshape(B, C, H, W, 2)


def _build_memcpy_module(reps=1):
    """Baseline: per-core DMA z -> out through SBUF (same traffic as kernel)."""
    nc = bacc.Bacc("TRN2", target_bir_lowering=False, debug=False,
                   detect_race_conditions=False)
    z_d = nc.dram_tensor("z", [B, M], f32, kind="ExternalInput").ap()
    out_d = nc.dram_tensor("out", [B, M], f32, kind="ExternalOutput").ap()
    z_r = z_d.rearrange("b (t j f) -> t j b f", t=NT, j=J, f=COLS)
    out_r = out_d.rearrange("b (t j f) -> t j b f", t=NT, j=J, f=COLS)
    with tile.TileContext(nc) as tc:
        with tc.tile_pool(name="buf", bufs=6) as buf:
            for _ in range(reps):
                for t in range(NT):
                    x = buf.tile([128, COLS], f32, tag="x")
                    nc.sync.dma_start(out=x[:], in_=z_r[t])
                    nc.scalar.dma_start(out=out_r[t], in_=x[:])
    nc.compile()
    return nc


def bench_memcpy(z, iters=10, reps=17):
    z = np.ascontiguousarray(z, dtype=np.float32)
    zr = z.reshape(B, C, H * W * 2)
    in_maps = []
    for c in range(NCORES):
        shard = np.ascontiguousarray(
            zr[:, c * C_PER:(c + 1) * C_PER].reshape(B, M))
        in_maps.append({"z": shard})
    ta, tb = bench_pair((_build_memcpy_module(1), _build_memcpy_module(reps)),
                        in_maps, in_maps, iters=iters, rounds=4)
    slopes = sorted((b - a) / (reps - 1) for a, b in zip(ta, tb))
    return slopes[len(slopes) // 2]


def _make_runner(nc, n_cores):
    """Build (and cache) the sharded jit executable for an SPMD module."""
    import jax
    import jax.numpy as jnp
    from jax.sharding import Mesh, PartitionSpec
    from jax.experimental.shard_map import shard_map
    from concourse import bass2jax
    from concourse.bass2jax import _bass_exec_p, install_neuronx_cc_hook
    from concourse import mybir as _mb

    install_neuronx_cc_hook()
    partition_name = (nc.partition_id_tensor.name
                      if nc.partition_id_tensor else None)
    in_names, out_names, out_avals, zero_outs = [], [], [], []
    for alloc in nc.m.functions[0].allocations:
        if not isinstance(alloc, _mb.MemoryLocationSet):
            continue
        name = alloc.memorylocations[0].name
        if alloc.kind == "ExternalInput":
            if name != partition_name:
                in_names.append(name)
        elif alloc.kind == "ExternalOutput":
            shape = tuple(alloc.tensor_shape)
            dtype = _mb.dt.np(alloc.dtype)
            out_names.append(name)
            out_avals.append(jax.core.ShapedArray(shape, dtype))
            zero_outs.append(np.zeros(shape, dtype))
    n_params = len(in_names)
    n_outs = len(out_avals)
    all_in_names = in_names + out_names
    if partition_name is not None:
        all_in_names.append(partition_name)

    def _body(*args):
        operands = list(args)
        if partition_name is not None:
            operands.append(bass2jax.partition_id_tensor())
        outs = _bass_exec_p.bind(
            *operands,
            out_avals=tuple(out_avals),
            in_names=tuple(all_in_names),
            out_names=tuple(out_names),
            lowering_input_output_aliases=(),
            sim_require_finite=True,
            sim_require_nnan=True,
            nc=nc,
        )
        return tuple(outs)

    devices = jax.devices()[:n_cores]
    mesh = Mesh(np.asarray(devices), ("core",))
    donate = tuple(range(n_params, n_params + n_outs))
    sharded = jax.jit(
        shard_map(_body, mesh=mesh,
                  in_specs=(PartitionSpec("core"),) * (n_params + n_outs),
                  out_specs=(PartitionSpec("core"),) * n_outs,
                  check_rep=False),
        donate_argnums=donate, keep_unused=True,
    )
    from jax.sharding import NamedSharding
    shard0 = NamedSharding(mesh, PartitionSpec("core"))
    return {
        "sharded": sharded, "shard0": shard0, "in_names": in_names,
        "out_names": out_names, "out_avals": out_avals,
        "zero_outs": zero_outs, "n_cores": n_cores,
    }


_RUNNERS = {}


def _get_runner(key, nc, n_cores):
    if key not in _RUNNERS:
        _RUNNERS[key] = _make_runner(nc, n_cores)
    return _RUNNERS[key]


def _run_module(runner, in_maps):
    import jax
    n_cores = runner["n_cores"]
    concat_in = [
        jax.device_put(
            np.concatenate([np.asarray(m[name]) for m in in_maps], axis=0),
            runner["shard0"])
        for name in runner["in_names"]
    ]
    zeros = [
        jax.device_put(
            np.zeros((n_cores * z.shape[0], *z.shape[1:]), z.dtype),
            runner["shard0"])
        for z in runner["zero_outs"]
    ]
    outs = runner["sharded"](*concat_in, *zeros)
    jax.block_until_ready(outs)
    return [
        {name: np.asarray(outs[i]).reshape(
            n_cores, *runner["out_avals"][i].shape)[c]
         for i, name in enumerate(runner["out_names"])}
        for c in range(n_cores)
    ]


def bench_module(nc, in_maps, iters=12, key=None):
    """Min-of-per-call timing of an SPMD bass module via the PJRT path."""
    import time as _time
    import jax
    runner = _make_runner(nc, len(in_maps))
    n_cores = runner["n_cores"]
    shard0 = runner["shard0"]
    sharded = runner["sharded"]
    concat_in = [
        jax.device_put(
            np.concatenate([np.asarray(m[name]) for m in in_maps], axis=0),
            shard0)
        for name in runner["in_names"]
    ]
    zero_sets = []
    for _ in range(iters + 1):
        zero_sets.append([
            jax.device_put(
                np.zeros((n_cores * z.shape[0], *z.shape[1:]), z.dtype),
                shard0)
            for z in runner["zero_outs"]
        ])
    outs = sharded(*concat_in, *zero_sets[0])
    jax.block_until_ready(outs)

    def one_batch(ks):
        t0 = _time.perf_counter()
        last = None
        for k in ks:
            last = sharded(*concat_in, *zero_sets[k + 1])
        jax.block_until_ready(last)
        return (_time.perf_counter() - t0) / len(ks), last

    dt, last = one_batch(range(iters))
    results = [
        {name: np.asarray(last[i]).reshape(
            n_cores, *runner["out_avals"][i].shape)[c]
         for i, name in enumerate(runner["out_names"])}
        for c in range(n_cores)
    ]
    return dt * 1e9, results


def bench_pair(ncs, in_maps_a, in_maps_b, iters=8, rounds=4):
    """Interleaved async-batch timing of two modules; returns
    (median per-call ns A, median per-call ns B, per-round lists)."""
    import time as _time
    import jax
    runners = [_get_runner(("m", id(ncs[0])), ncs[0], len(in_maps_a)),
               _get_runner(("m", id(ncs[1])), ncs[1], len(in_maps_b))]
    sides = []
    for runner, im in ((runners[0], in_maps_a), (runners[1], in_maps_b)):
        concat_in = [
            jax.device_put(
                np.concatenate([np.asarray(m[name]) for m in im], axis=0),
                runner["shard0"])
            for name in runner["in_names"]
        ]
        n_cores = runner["n_cores"]
        zsets = []
        for _ in range(iters * rounds + 1):
            zsets.append([
                jax.device_put(
                    np.zeros((n_cores * z.shape[0], *z.shape[1:]), z.dtype),
                    runner["shard0"])
                for z in runner["zero_outs"]
            ])
        sides.append((runner, concat_in, zsets))
        out = runner["sharded"](*concat_in, *zsets[0])
        jax.block_until_ready(out)
    ta, tb = [], []
    k = [0, 0]
    for r in range(rounds):
        for side, rec in ((0, ta), (1, tb)):
            runner, concat_in, zsets = sides[side]
            t0 = _time.perf_counter()
            last = None
            for _ in range(iters):
                k[side] += 1
                last = runner["sharded"](*concat_in, *zsets[k[side]])
            jax.block_until_ready(last)
            rec.append((_time.perf_counter() - t0) / iters * 1e9)
    return ta, tb


def bench(z, gamma, beta, iters=10, reps=17, with_memcpy=False):
    """Slope-based device timing: time modules with `reps`=1 and `reps`=R
    internal repetitions of the full pipeline; per-kernel device time =
    (t_R - t_1) / (R - 1), which cancels the per-dispatch axon overhead."""
    in_maps = _in_maps_for(z, gamma, beta)
    ta, tb = bench_pair((_get_module(1), _get_module(reps)),
                        in_maps, in_maps, iters=iters, rounds=4)
    slopes = sorted((b - a) / (reps - 1) for a, b in zip(ta, tb))
    ns = slopes[len(slopes) // 2]
    m1 = _get_module(1)
    runner = _get_runner(("m", id(m1)), m1, NCORES)
    results = _run_module(runner, in_maps)
    t1_ns, tR_ns = min(ta), min(tb)
    out = np.empty((B, C, H * W * 2), dtype=np.float32)
    for c in range(NCORES):
        out[:, c * C_PER:(c + 1) * C_PER] = results[c]["out"].reshape(
            B, C_PER, H * W * 2)
    return out.reshape(B, C, H, W, 2), ns, (t1_ns, tR_ns)


def run_traced(z, gamma, beta):
    """Like kernel() but with NTFF tracing; returns (output, exec_time_ns)."""
    in_maps = _in_maps_for(z, gamma, beta)
    nc = _get_module()
    res = run_bass_kernel_spmd(nc, in_maps, core_ids=list(range(NCORES)),
                               trace=True)
    out = np.empty((B, C, H * W * 2), dtype=np.float32)
    for c in range(NCORES):
        out[:, c * C_PER:(c + 1) * C_PER] = res.results[c]["out"].reshape(
            B, C_PER, H * W * 2)
    return out.reshape(B, C, H, W, 2), res.exec_time_ns, res
